# revision 2
# baseline (speedup 1.0000x reference)
"""CodeSwitchLoss Trainium2 kernel, v2: symmetric-half similarity matrix.

E = exp(S/t) is symmetric, so only the upper-triangular block pairs are
computed (~half the matmul + exp area of the data-parallel baseline). Each
computed block contributes its row-sums (activation accumulator) to its row
anchors and its column-sums (DVE column accumulation + gpsimd partition
all-reduce) to its mirrored row anchors. Per-anchor partial sums, the
same-sample diagonal entries (pos terms), and the column sums are DMA'd out;
the tiny nonlinear combine (logs + cs regularization) runs host-side in
fp64, exactly as the baseline already ran normalization host-side.

Work layout (uniform across cores; per-core differences live in host data
packing only): 8 sample-groups of 128; slabs = (version, group). Core q owns
group q: a triangle job over its own group (pairs a<=b), four cross jobs
(row slab a of q) x [3 following groups + a 256-wide half of the diametric
partner group]. A circulant tournament (+1,+2,+3) plus the half-split of the
diameter class covers each of the 528 unordered block pairs exactly once.
"""

import numpy as np
import ml_dtypes

B = 1024
D = 1024
P = 128
NV = 4
G = 8
NC_CORES = 8
KCH = D // P  # 8
WTOT = 20 * P  # 2560 cols: own(4) + 3 groups(12) + partner(4) slabs
INV_T = 10.0

CROSS_W = 1792  # 1536 (3 groups) + 256 (diameter half)
# triangle row offsets: rows packed 0,1,3,2 so no block crosses a 512-col
# PSUM bank boundary (row2's 256-wide block would straddle col 1024)
TRI_OFF = [0, 512, 1024, 896]
TRI_W = [512, 384, 256, 128]
DPAIRS = [(a, b) for a in range(NV) for b in range(a, NV)]  # 10

_compiled = {}


def _build_kernel():
    from contextlib import ExitStack

    import concourse.tile as tile
    from concourse import bacc, bass_isa, mybir

    fp32 = mybir.dt.float32
    bf16 = mybir.dt.bfloat16
    int32 = mybir.dt.int32
    fp8 = mybir.dt.float8e4
    AX = mybir.AxisListType
    ALU = mybir.AluOpType
    ACTF = mybir.ActivationFunctionType
    DR = mybir.MatmulPerfMode.DoubleRow

    nc = bacc.Bacc(
        "TRN2",
        target_bir_lowering=False,
        debug=False,
        enable_asserts=False,
        num_devices=NC_CORES,
    )
    cb_d = nc.dram_tensor("cb", [P, KCH * WTOT], fp8, kind="ExternalInput").ap()
    rs_d = nc.dram_tensor("rs", [P, 12], fp32, kind="ExternalOutput").ap()
    dv_d = nc.dram_tensor("dv", [P, 10], fp32, kind="ExternalOutput").ap()
    cs_d = nc.dram_tensor("cs", [1, 1408], fp32, kind="ExternalOutput").ap()
    cs2_d = nc.dram_tensor("cs2", [1, 1024], fp32, kind="ExternalOutput").ap()

    cbr = cb_d.rearrange("p (m w) -> p m w", m=KCH)

    with tile.TileContext(nc) as tc, ExitStack() as ctx:
        consts = ctx.enter_context(tc.tile_pool(name="consts", bufs=1))
        wpool = ctx.enter_context(tc.tile_pool(name="w", bufs=1))
        # PSUM budget (8 banks): psum_p 2 x [P,1536] (3 banks each) +
        # psumh_p 1 x [P,1024] (2 banks)
        psum_p = ctx.enter_context(tc.tile_pool(name="psum", bufs=2, space="PSUM"))
        psumh_p = ctx.enter_context(tc.tile_pool(name="psumh", bufs=1, space="PSUM"))
        esb_p = ctx.enter_context(tc.tile_pool(name="esb", bufs=4))
        acc_p = ctx.enter_context(tc.tile_pool(name="acc", bufs=1))
        dscr_p = ctx.enter_context(tc.tile_pool(name="dscr", bufs=2))

        # eye mask (bf16) built on device
        eye_i = consts.tile([P, P], int32, tag="eye_i")
        nc.gpsimd.iota(eye_i, pattern=[[1, P]], base=0, channel_multiplier=-1)
        eye_sb = consts.tile([P, P], bf16, tag="eye")
        nc.vector.tensor_scalar(
            out=eye_sb, in0=eye_i, scalar1=0, scalar2=None, op0=ALU.is_equal
        )

        # warm the Exp activation table at t=0 (only table we ever need)
        aw = consts.tile([P, 1], fp32, tag="actwarm")
        nc.vector.memset(aw, 0.0)
        nc.scalar.activation(out=aw, in_=aw, func=ACTF.Exp)

        # PE p-state keep-alive during the DMA staircase (paced on Pool so
        # DVE stays free for the real reduction work)
        wsb = consts.tile([P, P], bf16, tag="wsb")
        nc.vector.memset(wsb, 0.0)
        ones1 = consts.tile([P, 1], bf16, tag="ones1")
        nc.vector.memset(ones1, 1.0)
        ka_ps = psum_p.tile([P, 1536], fp32, tag="ps", name="ka_ps")
        nc.tensor.matmul(ka_ps[:, 0:16], wsb, wsb[:, 0:16], start=True, stop=True)
        ka_big = consts.tile([P, 900], fp32, tag="ka_big")
        ka_mv = consts.tile([P, 4, 4], bf16, tag="ka_mv")
        for kk in range(4):
            nc.gpsimd.memset(ka_big, 0.0)
            nc.gpsimd.memset(ka_mv[:, kk, :], 0.0)
            nc.tensor.matmul(
                ka_ps[:, 0:4], wsb, ka_mv[:, kk, :], start=True, stop=True
            )

        # ---- loads: own group, partner halves, then the 3 out-groups ----
        colbuf = wpool.tile([P, KCH, WTOT], fp8, tag="colbuf")
        for s in (0, 2048, 512, 1024, 1536):
            nc.sync.dma_start(
                out=colbuf[:, :, s : s + 512], in_=cbr[:, :, s : s + 512]
            )

        rsums = acc_p.tile([P, 12], fp32, tag="rsums")
        dvals = acc_p.tile([P, 10], fp32, tag="dvals")
        cspack = acc_p.tile([P, 1408], fp32, tag="cspack")

        def chain(tgt, row, c0, c1):
            """PSUM-accumulating fp8 DoubleRow chains: rows = slab `row`,
            cols = colbuf[c0:c1], tiled to the 512-col PSUM-bank limit."""
            for t0 in range(0, c1 - c0, 512):
                t1 = min(t0 + 512, c1 - c0)
                for m in range(0, KCH, 2):
                    nc.tensor.matmul(
                        tgt[:, t0:t1],
                        colbuf[:, m : m + 2, row * P : (row + 1) * P],
                        colbuf[:, m : m + 2, c0 + t0 : c0 + t1],
                        start=(m == 0),
                        stop=(m == KCH - 2),
                        perf_mode=DR,
                    )

        # ---- triangle job (own group, pairs a<=b) ----
        ps_tri = psum_p.tile([P, 1536], fp32, tag="ps", name="ps_tri")
        for a in range(NV):
            chain(ps_tri[:, TRI_OFF[a] : TRI_OFF[a] + TRI_W[a]], a, a * P, 512)
        e_tri = consts.tile([P, 1280], bf16, tag="e_tri")
        nc.scalar.activation(
            out=e_tri, in_=ps_tri[:, 0:1280], func=ACTF.Exp, scale=INV_T
        )
        # per-row rowsums of the triangle (DVE)
        for a in range(NV):
            nc.vector.reduce_sum(
                out=rsums[:, 4 + a : 5 + a],
                in_=e_tri[:, TRI_OFF[a] : TRI_OFF[a] + TRI_W[a]],
                axis=AX.X,
            )
        # same-sample diagonal entries (pos terms)
        for idx, (a, bb) in enumerate(DPAIRS):
            o = TRI_OFF[a] + (bb - a) * P
            dscr = dscr_p.tile([P, P], bf16, tag="dscr", name="dscr")
            nc.vector.scalar_tensor_tensor(
                out=dscr, in0=e_tri[:, o : o + P], scalar=1.0, in1=eye_sb,
                op0=ALU.mult, op1=ALU.mult,
                accum_out=dvals[:, idx : idx + 1],
            )
        # own-group column sums from the triangle's b>a blocks
        caccown = acc_p.tile([P, 384], bf16, tag="caccown")
        nc.vector.tensor_copy(out=caccown, in_=e_tri[:, 128:512])
        nc.vector.tensor_add(caccown[:, 128:384], caccown[:, 128:384],
                             e_tri[:, 640:896])
        nc.vector.tensor_add(caccown[:, 256:384], caccown[:, 256:384],
                             e_tri[:, 1152:1280])

        # ---- diameter-half blocks (need only own + partner pieces, which
        # load first: this fills the activation engine before the out-groups
        # arrive) ----
        cacch = acc_p.tile([P, 2, 256], bf16, tag="cacch")
        ps_half = psumh_p.tile([P, 1024], fp32, tag="psh", name="ps_half")
        ehalf = consts.tile([P, NV, 256], bf16, tag="ehalf")
        for k in range(NV):
            hs = 2048 if k < 2 else 2304
            chain(ps_half[:, k * 256 : (k + 1) * 256], k, hs, hs + 256)
        for k in range(NV):
            nc.scalar.activation(
                out=ehalf[:, k, :], in_=ps_half[:, k * 256 : (k + 1) * 256],
                func=ACTF.Exp, scale=INV_T,
                accum_out=rsums[:, 8 + k : 9 + k],
            )
        nc.vector.tensor_add(cacch[:, 0, :], ehalf[:, 0, :], ehalf[:, 1, :])
        nc.vector.tensor_add(cacch[:, 1, :], ehalf[:, 2, :], ehalf[:, 3, :])

        # ---- cross jobs k = 0..3 over the three out-groups ----
        caccg = acc_p.tile([P, 3, 512], bf16, tag="caccg")
        etiles = {}
        for k in range(NV):
            ps = psum_p.tile([P, 1536], fp32, tag="ps", name="psc")
            chain(ps, k, 512, 2048)
            ek = esb_p.tile([P, 1536], bf16, tag="ek", name="ek")
            nc.scalar.activation(
                out=ek, in_=ps, func=ACTF.Exp, scale=INV_T,
                accum_out=rsums[:, k : k + 1],
            )
            etiles[k] = ek
            if k == 1:
                for j in range(3):
                    nc.vector.tensor_add(
                        caccg[:, j, :],
                        etiles[0][:, j * 512 : (j + 1) * 512],
                        etiles[1][:, j * 512 : (j + 1) * 512],
                    )
            elif k >= 2:
                for j in range(3):
                    nc.vector.tensor_add(
                        caccg[:, j, :], caccg[:, j, :],
                        etiles[k][:, j * 512 : (j + 1) * 512],
                    )

        # ---- column sums: halves + own + group o3 on gpsimd, groups
        # o1/o2 as ones-matmuls on the (idle by then) PE ----
        nc.gpsimd.partition_all_reduce(
            cspack[:, 0:256], cacch[:, 0, :], channels=P,
            reduce_op=bass_isa.ReduceOp.add,
        )
        nc.gpsimd.partition_all_reduce(
            cspack[:, 256:512], cacch[:, 1, :], channels=P,
            reduce_op=bass_isa.ReduceOp.add,
        )
        nc.gpsimd.partition_all_reduce(
            cspack[:, 512:896], caccown, channels=P,
            reduce_op=bass_isa.ReduceOp.add,
        )
        nc.gpsimd.partition_all_reduce(
            cspack[:, 896:1408], caccg[:, 2, :], channels=P,
            reduce_op=bass_isa.ReduceOp.add,
        )
        cs2_ps = psumh_p.tile([P, 1024], fp32, tag="psh", name="cs2_ps")
        for j in range(2):
            nc.tensor.matmul(
                cs2_ps[0:1, j * 512 : (j + 1) * 512],
                ones1, caccg[:, j, :], start=True, stop=True,
            )
        cs2_sb = acc_p.tile([1, 1024], fp32, tag="cs2_sb")
        nc.scalar.copy(out=cs2_sb, in_=cs2_ps[0:1, :])

        nc.sync.dma_start(out=rs_d, in_=rsums)
        nc.sync.dma_start(out=dv_d, in_=dvals)
        nc.sync.dma_start(out=cs_d, in_=cspack[0:1, :])
        nc.sync.dma_start(out=cs2_d, in_=cs2_sb)

    nc.compile()
    return nc


def _get_nc():
    if "nc" not in _compiled:
        _compiled["nc"] = _build_kernel()
    return _compiled["nc"]


def _core_slabs(q):
    """Per-core slab order: own group, 3 out-groups, partner halves."""
    order = [(v, q) for v in range(NV)]
    for d in (1, 2, 3):
        order += [(v, (q + d) % G) for v in range(NV)]
    p = (q + 4) % G
    halves = [0, 1, 2, 3] if q < 4 else [2, 3, 0, 1]
    order += [(v, p) for v in halves]
    return order


def _make_in_maps(english, etok, ktoe, korean, cs_ratios):
    V4f = np.stack([
        np.asarray(english, dtype=np.float32),
        np.asarray(korean, dtype=np.float32),
        np.asarray(etok, dtype=np.float32),
        np.asarray(ktoe, dtype=np.float32),
    ])  # [4, B, D], reference version order [e, k, etk, kte]
    V4f = V4f / np.linalg.norm(V4f, axis=2, keepdims=True)
    QT = np.ascontiguousarray(V4f.transpose(0, 2, 1)).astype(
        ml_dtypes.float8_e4m3
    )  # [4, D, B]

    in_maps = []
    for q in range(NC_CORES):
        cbuf = np.empty((P, KCH, WTOT), dtype=ml_dtypes.float8_e4m3)
        for j, (v, g) in enumerate(_core_slabs(q)):
            blk = QT[v][:, g * P : (g + 1) * P]  # [D, 128]
            cbuf[:, :, j * P : (j + 1) * P] = blk.reshape(
                KCH, P, P
            ).transpose(1, 0, 2)
        in_maps.append({"cb": cbuf.reshape(P, KCH * WTOT)})
    return in_maps, V4f


def kernel(english, etok, ktoe, korean, cs_ratios):
    from concourse.bass_utils import run_bass_kernel_spmd

    in_maps, V4f = _make_in_maps(english, etok, ktoe, korean, cs_ratios)
    nc = _get_nc()
    res = run_bass_kernel_spmd(nc, in_maps, core_ids=list(range(NC_CORES)))

    R = np.zeros((NV, B), dtype=np.float64)  # full-row sums per anchor
    dall = np.zeros((NV, NV, B), dtype=np.float64)  # same-sample exp entries
    for q, rmap in enumerate(res.results):
        rs = rmap["rs"].astype(np.float64)
        dv = rmap["dv"].astype(np.float64)
        cs = rmap["cs"].astype(np.float64)[0]
        cs2 = rmap["cs2"].astype(np.float64)[0]
        sl = slice(q * P, (q + 1) * P)
        for k in range(NV):
            # cross main + triangle + diameter-half rowsum partials
            R[k, sl] += rs[:, k] + rs[:, 4 + k] + rs[:, 8 + k]
        for idx, (a, bb) in enumerate(DPAIRS):
            dall[a, bb, q * P : (q + 1) * P] = dv[:, idx]
            dall[bb, a, q * P : (q + 1) * P] = dv[:, idx]
        # column-sum contributions -> mirrored rows
        for d in (1, 2, 3):
            g = (q + d) % G
            piece = cs2[(d - 1) * 512 : d * 512] if d < 3 else cs[896:1408]
            for v in range(NV):
                R[v, g * P : (g + 1) * P] += piece[v * P : (v + 1) * P]
        slabs = _core_slabs(q)
        ph = slabs[16:20]  # partner halves in core order
        for hh in range(2):
            piece = cs[hh * 256 : (hh + 1) * 256]
            for i in range(2):
                v, g = ph[hh * 2 + i]
                R[v, g * P : (g + 1) * P] += piece[i * P : (i + 1) * P]
        own = cs[512:896]
        for bb in range(1, NV):
            R[bb, sl] += own[(bb - 1) * P : bb * P]

    same_i = dall.sum(axis=1)  # [4, B]
    self_e = np.einsum("aab->ab", dall)
    pos = same_i - self_e
    tot = R - self_e  # pos + neg
    contrastive = (np.log(tot) - np.log(pos)).sum()

    r = np.asarray(cs_ratios, dtype=np.float64)[:, None]
    e, k64, etk, kte = (V4f[i].astype(np.float64) for i in range(4))
    reg = (
        np.linalg.norm(etk - (r * e + (1 - r) * k64), axis=1)
        + np.linalg.norm(kte - ((1 - r) * e + r * k64), axis=1)
    ).sum()

    return np.float32((contrastive + 0.5 * reg) / B)


# revision 3
# speedup vs baseline: 1.0272x; 1.0272x over previous
"""CodeSwitchLoss Trainium2 kernel, v2: symmetric-half similarity matrix.

E = exp(S/t) is symmetric, so only the upper-triangular block pairs are
computed (~half the matmul + exp area of the data-parallel baseline). Each
computed block contributes its row-sums (activation accumulator) to its row
anchors and its column-sums (DVE column accumulation + gpsimd partition
all-reduce) to its mirrored row anchors. Per-anchor partial sums, the
same-sample diagonal entries (pos terms), and the column sums are DMA'd out;
the tiny nonlinear combine (logs + cs regularization) runs host-side in
fp64, exactly as the baseline already ran normalization host-side.

Work layout (uniform across cores; per-core differences live in host data
packing only): 8 sample-groups of 128; slabs = (version, group). Core q owns
group q: a triangle job over its own group (pairs a<=b), four cross jobs
(row slab a of q) x [3 following groups + a 256-wide half of the diametric
partner group]. A circulant tournament (+1,+2,+3) plus the half-split of the
diameter class covers each of the 528 unordered block pairs exactly once.
"""

import numpy as np
import ml_dtypes

B = 1024
D = 1024
P = 128
NV = 4
G = 8
NC_CORES = 8
KCH = D // P  # 8
WTOT = 20 * P  # 2560 cols: own(4) + 3 groups(12) + partner(4) slabs
INV_T = 10.0

CROSS_W = 1792  # 1536 (3 groups) + 256 (diameter half)
# triangle row offsets: rows packed 0,1,3,2 so no block crosses a 512-col
# PSUM bank boundary (row2's 256-wide block would straddle col 1024)
TRI_OFF = [0, 512, 1024, 896]
TRI_W = [512, 384, 256, 128]
DPAIRS = [(a, b) for a in range(NV) for b in range(a, NV)]  # 10

_compiled = {}


def _build_kernel():
    from contextlib import ExitStack

    import concourse.tile as tile
    from concourse import bacc, bass_isa, mybir

    fp32 = mybir.dt.float32
    bf16 = mybir.dt.bfloat16
    int32 = mybir.dt.int32
    fp8 = mybir.dt.float8e4
    AX = mybir.AxisListType
    ALU = mybir.AluOpType
    ACTF = mybir.ActivationFunctionType
    DR = mybir.MatmulPerfMode.DoubleRow

    nc = bacc.Bacc(
        "TRN2",
        target_bir_lowering=False,
        debug=False,
        enable_asserts=False,
        num_devices=NC_CORES,
    )
    cb_d = nc.dram_tensor("cb", [P, KCH * WTOT], fp8, kind="ExternalInput").ap()
    rs_d = nc.dram_tensor("rs", [P, 12], fp32, kind="ExternalOutput").ap()
    dv_d = nc.dram_tensor("dv", [P, 10], fp32, kind="ExternalOutput").ap()
    cs_d = nc.dram_tensor("cs", [1, 1920], fp32, kind="ExternalOutput").ap()
    cs2_d = nc.dram_tensor("cs2", [1, 1024], fp32, kind="ExternalOutput").ap()

    cbr = cb_d.rearrange("p (m w) -> p m w", m=KCH)

    with tile.TileContext(nc) as tc, ExitStack() as ctx:
        consts = ctx.enter_context(tc.tile_pool(name="consts", bufs=1))
        wpool = ctx.enter_context(tc.tile_pool(name="w", bufs=1))
        # PSUM budget (8 banks): psum_p 2 x [P,1536] (3 banks each) +
        # psumh_p 1 x [P,1024] (2 banks)
        psum_p = ctx.enter_context(tc.tile_pool(name="psum", bufs=2, space="PSUM"))
        psumh_p = ctx.enter_context(tc.tile_pool(name="psumh", bufs=1, space="PSUM"))
        esb_p = ctx.enter_context(tc.tile_pool(name="esb", bufs=4))
        acc_p = ctx.enter_context(tc.tile_pool(name="acc", bufs=1))
        dscr_p = ctx.enter_context(tc.tile_pool(name="dscr", bufs=2))

        # eye mask (bf16) built on device
        eye_i = consts.tile([P, P], int32, tag="eye_i")
        nc.gpsimd.iota(eye_i, pattern=[[1, P]], base=0, channel_multiplier=-1)
        eye_sb = consts.tile([P, P], bf16, tag="eye")
        nc.vector.tensor_scalar(
            out=eye_sb, in0=eye_i, scalar1=0, scalar2=None, op0=ALU.is_equal
        )

        # warm the Exp activation table at t=0 (only table we ever need)
        aw = consts.tile([P, 1], fp32, tag="actwarm")
        nc.vector.memset(aw, 0.0)
        nc.scalar.activation(out=aw, in_=aw, func=ACTF.Exp)

        # PE p-state keep-alive during the DMA staircase (paced on Pool so
        # DVE stays free for the real reduction work)
        wsb = consts.tile([P, P], bf16, tag="wsb")
        nc.vector.memset(wsb, 0.0)
        ones1 = consts.tile([P, 1], bf16, tag="ones1")
        nc.vector.memset(ones1, 1.0)
        ka_ps = psum_p.tile([P, 1536], fp32, tag="ps", name="ka_ps")
        nc.tensor.matmul(ka_ps[:, 0:16], wsb, wsb[:, 0:16], start=True, stop=True)
        ka_big = consts.tile([P, 900], fp32, tag="ka_big")
        ka_mv = consts.tile([P, 4, 4], bf16, tag="ka_mv")
        for kk in range(4):
            nc.gpsimd.memset(ka_big, 0.0)
            nc.gpsimd.memset(ka_mv[:, kk, :], 0.0)
            nc.tensor.matmul(
                ka_ps[:, 0:4], wsb, ka_mv[:, kk, :], start=True, stop=True
            )

        # ---- loads: own group, partner halves, then the 3 out-groups ----
        colbuf = wpool.tile([P, KCH, WTOT], fp8, tag="colbuf")
        for s in (0, 2048, 512, 1024, 1536):
            nc.sync.dma_start(
                out=colbuf[:, :, s : s + 512], in_=cbr[:, :, s : s + 512]
            )

        rsums = acc_p.tile([P, 12], fp32, tag="rsums")
        dvals = acc_p.tile([P, 10], fp32, tag="dvals")
        cspack = acc_p.tile([P, 1920], fp32, tag="cspack")

        def chain(tgt, row, c0, c1):
            """PSUM-accumulating fp8 DoubleRow chains: rows = slab `row`,
            cols = colbuf[c0:c1], tiled to the 512-col PSUM-bank limit."""
            for t0 in range(0, c1 - c0, 512):
                t1 = min(t0 + 512, c1 - c0)
                for m in range(0, KCH, 2):
                    nc.tensor.matmul(
                        tgt[:, t0:t1],
                        colbuf[:, m : m + 2, row * P : (row + 1) * P],
                        colbuf[:, m : m + 2, c0 + t0 : c0 + t1],
                        start=(m == 0),
                        stop=(m == KCH - 2),
                        perf_mode=DR,
                    )

        # ---- triangle job (own group, pairs a<=b) ----
        ps_tri = psum_p.tile([P, 1536], fp32, tag="ps", name="ps_tri")
        for a in range(NV):
            chain(ps_tri[:, TRI_OFF[a] : TRI_OFF[a] + TRI_W[a]], a, a * P, 512)
        e_tri = consts.tile([P, 1280], bf16, tag="e_tri")
        nc.scalar.activation(
            out=e_tri, in_=ps_tri[:, 0:1280], func=ACTF.Exp, scale=INV_T
        )
        # per-row rowsums of the triangle (DVE)
        for a in range(NV):
            nc.vector.reduce_sum(
                out=rsums[:, 4 + a : 5 + a],
                in_=e_tri[:, TRI_OFF[a] : TRI_OFF[a] + TRI_W[a]],
                axis=AX.X,
            )
        # same-sample diagonal entries (pos terms)
        for idx, (a, bb) in enumerate(DPAIRS):
            o = TRI_OFF[a] + (bb - a) * P
            dscr = dscr_p.tile([P, P], bf16, tag="dscr", name="dscr")
            nc.vector.scalar_tensor_tensor(
                out=dscr, in0=e_tri[:, o : o + P], scalar=1.0, in1=eye_sb,
                op0=ALU.mult, op1=ALU.mult,
                accum_out=dvals[:, idx : idx + 1],
            )
        # own-group column sums from the triangle's b>a blocks
        caccown = acc_p.tile([P, 384], bf16, tag="caccown")
        nc.vector.tensor_copy(out=caccown, in_=e_tri[:, 128:512])
        nc.vector.tensor_add(caccown[:, 128:384], caccown[:, 128:384],
                             e_tri[:, 640:896])
        nc.vector.tensor_add(caccown[:, 256:384], caccown[:, 256:384],
                             e_tri[:, 1152:1280])

        # ---- diameter-half blocks (need only own + partner pieces, which
        # load first: this fills the activation engine before the out-groups
        # arrive) ----
        cacch = acc_p.tile([P, 2, 256], bf16, tag="cacch")
        ps_half = psumh_p.tile([P, 1024], fp32, tag="psh", name="ps_half")
        ehalf = consts.tile([P, NV, 256], bf16, tag="ehalf")
        for k in range(NV):
            hs = 2048 if k < 2 else 2304
            chain(ps_half[:, k * 256 : (k + 1) * 256], k, hs, hs + 256)
        for k in range(NV):
            nc.scalar.activation(
                out=ehalf[:, k, :], in_=ps_half[:, k * 256 : (k + 1) * 256],
                func=ACTF.Exp, scale=INV_T,
                accum_out=rsums[:, 8 + k : 9 + k],
            )
        nc.vector.tensor_add(cacch[:, 0, :], ehalf[:, 0, :], ehalf[:, 1, :])
        nc.vector.tensor_add(cacch[:, 1, :], ehalf[:, 2, :], ehalf[:, 3, :])

        # ---- cross jobs k = 0..3 over the three out-groups ----
        caccg = acc_p.tile([P, 3, 512], bf16, tag="caccg")
        etiles = {}
        for k in range(NV):
            ps = psum_p.tile([P, 1536], fp32, tag="ps", name="psc")
            chain(ps, k, 512, 2048)
            ek = esb_p.tile([P, 1536], bf16, tag="ek", name="ek")
            nc.scalar.activation(
                out=ek, in_=ps, func=ACTF.Exp, scale=INV_T,
                accum_out=rsums[:, k : k + 1],
            )
            etiles[k] = ek
            # accumulate only E0..E2; E3's column contribution joins via a
            # chained ones-matmul / separate Pool reduce so nothing but the
            # final reductions trail the last activation
            if k == 1:
                for j in range(3):
                    nc.vector.tensor_add(
                        caccg[:, j, :],
                        etiles[0][:, j * 512 : (j + 1) * 512],
                        etiles[1][:, j * 512 : (j + 1) * 512],
                    )
            elif k == 2:
                for j in range(3):
                    nc.vector.tensor_add(
                        caccg[:, j, :], caccg[:, j, :],
                        etiles[k][:, j * 512 : (j + 1) * 512],
                    )

        # ---- column sums: halves + own + group o3 on gpsimd, groups
        # o1/o2 as ones-matmuls on the (idle by then) PE ----
        nc.gpsimd.partition_all_reduce(
            cspack[:, 0:256], cacch[:, 0, :], channels=P,
            reduce_op=bass_isa.ReduceOp.add,
        )
        nc.gpsimd.partition_all_reduce(
            cspack[:, 256:512], cacch[:, 1, :], channels=P,
            reduce_op=bass_isa.ReduceOp.add,
        )
        nc.gpsimd.partition_all_reduce(
            cspack[:, 512:896], caccown, channels=P,
            reduce_op=bass_isa.ReduceOp.add,
        )
        nc.gpsimd.partition_all_reduce(
            cspack[:, 896:1408], caccg[:, 2, :], channels=P,
            reduce_op=bass_isa.ReduceOp.add,
        )
        nc.gpsimd.partition_all_reduce(
            cspack[:, 1408:1920], etiles[3][:, 1024:1536], channels=P,
            reduce_op=bass_isa.ReduceOp.add,
        )
        cs2_ps = psumh_p.tile([P, 1024], fp32, tag="psh", name="cs2_ps")
        for j in range(2):
            nc.tensor.matmul(
                cs2_ps[0:1, j * 512 : (j + 1) * 512],
                ones1, caccg[:, j, :], start=True, stop=False,
            )
            nc.tensor.matmul(
                cs2_ps[0:1, j * 512 : (j + 1) * 512],
                ones1, etiles[3][:, j * 512 : (j + 1) * 512],
                start=False, stop=True,
            )
        cs2_sb = acc_p.tile([1, 1024], fp32, tag="cs2_sb")
        nc.scalar.copy(out=cs2_sb[:, 0:512], in_=cs2_ps[0:1, 0:512])
        nc.vector.tensor_copy(out=cs2_sb[:, 512:1024], in_=cs2_ps[0:1, 512:1024])

        nc.sync.dma_start(out=rs_d, in_=rsums)
        nc.sync.dma_start(out=dv_d, in_=dvals)
        nc.sync.dma_start(out=cs_d, in_=cspack[0:1, :])
        nc.sync.dma_start(out=cs2_d, in_=cs2_sb)

    nc.compile()
    return nc


def _get_nc():
    if "nc" not in _compiled:
        _compiled["nc"] = _build_kernel()
    return _compiled["nc"]


def _core_slabs(q):
    """Per-core slab order: own group, 3 out-groups, partner halves."""
    order = [(v, q) for v in range(NV)]
    for d in (1, 2, 3):
        order += [(v, (q + d) % G) for v in range(NV)]
    p = (q + 4) % G
    halves = [0, 1, 2, 3] if q < 4 else [2, 3, 0, 1]
    order += [(v, p) for v in halves]
    return order


def _make_in_maps(english, etok, ktoe, korean, cs_ratios):
    V4f = np.stack([
        np.asarray(english, dtype=np.float32),
        np.asarray(korean, dtype=np.float32),
        np.asarray(etok, dtype=np.float32),
        np.asarray(ktoe, dtype=np.float32),
    ])  # [4, B, D], reference version order [e, k, etk, kte]
    V4f = V4f / np.linalg.norm(V4f, axis=2, keepdims=True)
    QT = np.ascontiguousarray(V4f.transpose(0, 2, 1)).astype(
        ml_dtypes.float8_e4m3
    )  # [4, D, B]

    in_maps = []
    for q in range(NC_CORES):
        cbuf = np.empty((P, KCH, WTOT), dtype=ml_dtypes.float8_e4m3)
        for j, (v, g) in enumerate(_core_slabs(q)):
            blk = QT[v][:, g * P : (g + 1) * P]  # [D, 128]
            cbuf[:, :, j * P : (j + 1) * P] = blk.reshape(
                KCH, P, P
            ).transpose(1, 0, 2)
        in_maps.append({"cb": cbuf.reshape(P, KCH * WTOT)})
    return in_maps, V4f


def kernel(english, etok, ktoe, korean, cs_ratios):
    from concourse.bass_utils import run_bass_kernel_spmd

    in_maps, V4f = _make_in_maps(english, etok, ktoe, korean, cs_ratios)
    nc = _get_nc()
    res = run_bass_kernel_spmd(nc, in_maps, core_ids=list(range(NC_CORES)))

    R = np.zeros((NV, B), dtype=np.float64)  # full-row sums per anchor
    dall = np.zeros((NV, NV, B), dtype=np.float64)  # same-sample exp entries
    for q, rmap in enumerate(res.results):
        rs = rmap["rs"].astype(np.float64)
        dv = rmap["dv"].astype(np.float64)
        cs = rmap["cs"].astype(np.float64)[0]
        cs2 = rmap["cs2"].astype(np.float64)[0]
        sl = slice(q * P, (q + 1) * P)
        for k in range(NV):
            # cross main + triangle + diameter-half rowsum partials
            R[k, sl] += rs[:, k] + rs[:, 4 + k] + rs[:, 8 + k]
        for idx, (a, bb) in enumerate(DPAIRS):
            dall[a, bb, q * P : (q + 1) * P] = dv[:, idx]
            dall[bb, a, q * P : (q + 1) * P] = dv[:, idx]
        # column-sum contributions -> mirrored rows
        for d in (1, 2, 3):
            g = (q + d) % G
            piece = (cs2[(d - 1) * 512 : d * 512] if d < 3
                     else cs[896:1408] + cs[1408:1920])
            for v in range(NV):
                R[v, g * P : (g + 1) * P] += piece[v * P : (v + 1) * P]
        slabs = _core_slabs(q)
        ph = slabs[16:20]  # partner halves in core order
        for hh in range(2):
            piece = cs[hh * 256 : (hh + 1) * 256]
            for i in range(2):
                v, g = ph[hh * 2 + i]
                R[v, g * P : (g + 1) * P] += piece[i * P : (i + 1) * P]
        own = cs[512:896]
        for bb in range(1, NV):
            R[bb, sl] += own[(bb - 1) * P : bb * P]

    same_i = dall.sum(axis=1)  # [4, B]
    self_e = np.einsum("aab->ab", dall)
    pos = same_i - self_e
    tot = R - self_e  # pos + neg
    contrastive = (np.log(tot) - np.log(pos)).sum()

    r = np.asarray(cs_ratios, dtype=np.float64)[:, None]
    e, k64, etk, kte = (V4f[i].astype(np.float64) for i in range(4))
    reg = (
        np.linalg.norm(etk - (r * e + (1 - r) * k64), axis=1)
        + np.linalg.norm(kte - ((1 - r) * e + r * k64), axis=1)
    ).sum()

    return np.float32((contrastive + 0.5 * reg) / B)


# revision 4
# speedup vs baseline: 1.0549x; 1.0270x over previous
"""CodeSwitchLoss Trainium2 kernel, v2: symmetric-half similarity matrix.

E = exp(S/t) is symmetric, so only the upper-triangular block pairs are
computed (~half the matmul + exp area of the data-parallel baseline). Each
computed block contributes its row-sums (activation accumulator) to its row
anchors and its column-sums (DVE column accumulation + gpsimd partition
all-reduce) to its mirrored row anchors. Per-anchor partial sums, the
same-sample diagonal entries (pos terms), and the column sums are DMA'd out;
the tiny nonlinear combine (logs + cs regularization) runs host-side in
fp64, exactly as the baseline already ran normalization host-side.

Work layout (uniform across cores; per-core differences live in host data
packing only): 8 sample-groups of 128; slabs = (version, group). Core q owns
group q: a triangle job over its own group (pairs a<=b), four cross jobs
(row slab a of q) x [3 following groups + a 256-wide half of the diametric
partner group]. A circulant tournament (+1,+2,+3) plus the half-split of the
diameter class covers each of the 528 unordered block pairs exactly once.
"""

import numpy as np
import ml_dtypes

B = 1024
D = 1024
P = 128
NV = 4
G = 8
NC_CORES = 8
KCH = D // P  # 8
WTOT = 20 * P  # 2560 cols: own(4) + 3 groups(12) + partner(4) slabs
INV_T = 10.0

CROSS_W = 1792  # 1536 (3 groups) + 256 (diameter half)
# triangle row offsets: rows packed 0,1,3,2 so no block crosses a 512-col
# PSUM bank boundary (row2's 256-wide block would straddle col 1024)
TRI_OFF = [0, 512, 1024, 896]
TRI_W = [512, 384, 256, 128]
DPAIRS = [(a, b) for a in range(NV) for b in range(a, NV)]  # 10

_compiled = {}


def _build_kernel():
    from contextlib import ExitStack

    import concourse.tile as tile
    from concourse import bacc, bass_isa, mybir

    fp32 = mybir.dt.float32
    bf16 = mybir.dt.bfloat16
    int32 = mybir.dt.int32
    fp8 = mybir.dt.float8e4
    AX = mybir.AxisListType
    ALU = mybir.AluOpType
    ACTF = mybir.ActivationFunctionType
    DR = mybir.MatmulPerfMode.DoubleRow

    nc = bacc.Bacc(
        "TRN2",
        target_bir_lowering=False,
        debug=False,
        enable_asserts=False,
        num_devices=NC_CORES,
    )
    cb_d = nc.dram_tensor("cb", [P, KCH * WTOT], fp8, kind="ExternalInput").ap()
    rs_d = nc.dram_tensor("rs", [P, 12], fp32, kind="ExternalOutput").ap()
    dv_d = nc.dram_tensor("dv", [P, 10], fp32, kind="ExternalOutput").ap()
    cs_d = nc.dram_tensor("cs", [1, 2944], fp32, kind="ExternalOutput").ap()

    cbr = cb_d.rearrange("p (m w) -> p m w", m=KCH)

    with tile.TileContext(nc) as tc, ExitStack() as ctx:
        consts = ctx.enter_context(tc.tile_pool(name="consts", bufs=1))
        wpool = ctx.enter_context(tc.tile_pool(name="w", bufs=1))
        # PSUM budget (8 banks): psum_p 2 x [P,1536] (3 banks each) +
        # psumh_p 1 x [P,1024] (2 banks)
        psum_p = ctx.enter_context(tc.tile_pool(name="psum", bufs=2, space="PSUM"))
        psumh_p = ctx.enter_context(tc.tile_pool(name="psumh", bufs=1, space="PSUM"))
        esb_p = ctx.enter_context(tc.tile_pool(name="esb", bufs=4))
        acc_p = ctx.enter_context(tc.tile_pool(name="acc", bufs=1))
        dscr_p = ctx.enter_context(tc.tile_pool(name="dscr", bufs=2))

        # eye mask (bf16) built on device
        eye_i = consts.tile([P, P], int32, tag="eye_i")
        nc.gpsimd.iota(eye_i, pattern=[[1, P]], base=0, channel_multiplier=-1)
        eye_sb = consts.tile([P, P], bf16, tag="eye")
        nc.vector.tensor_scalar(
            out=eye_sb, in0=eye_i, scalar1=0, scalar2=None, op0=ALU.is_equal
        )

        # warm the Exp activation table at t=0 (only table we ever need)
        aw = consts.tile([P, 1], fp32, tag="actwarm")
        nc.vector.memset(aw, 0.0)
        nc.scalar.activation(out=aw, in_=aw, func=ACTF.Exp)

        # PE p-state keep-alive during the DMA staircase (paced on Pool so
        # DVE stays free for the real reduction work)
        wsb = consts.tile([P, P], bf16, tag="wsb")
        nc.vector.memset(wsb, 0.0)
        ones1 = consts.tile([P, 1], bf16, tag="ones1")
        nc.vector.memset(ones1, 1.0)
        ka_ps = psum_p.tile([P, 1536], fp32, tag="ps", name="ka_ps")
        nc.tensor.matmul(ka_ps[:, 0:16], wsb, wsb[:, 0:16], start=True, stop=True)
        ka_big = consts.tile([P, 900], fp32, tag="ka_big")
        ka_mv = consts.tile([P, 4, 4], bf16, tag="ka_mv")
        for kk in range(4):
            nc.gpsimd.memset(ka_big, 0.0)
            nc.gpsimd.memset(ka_mv[:, kk, :], 0.0)
            nc.tensor.matmul(
                ka_ps[:, 0:4], wsb, ka_mv[:, kk, :], start=True, stop=True
            )

        # ---- loads: own group, partner halves, then the 3 out-groups ----
        colbuf = wpool.tile([P, KCH, WTOT], fp8, tag="colbuf")
        for s in (0, 2048, 512, 1024, 1536):
            nc.sync.dma_start(
                out=colbuf[:, :, s : s + 512], in_=cbr[:, :, s : s + 512]
            )

        rsums = acc_p.tile([P, 12], fp32, tag="rsums")
        dvals = acc_p.tile([P, 10], fp32, tag="dvals")
        cspack = acc_p.tile([P, 2944], fp32, tag="cspack")

        def chain(tgt, row, c0, c1):
            """PSUM-accumulating fp8 DoubleRow chains: rows = slab `row`,
            cols = colbuf[c0:c1], tiled to the 512-col PSUM-bank limit."""
            for t0 in range(0, c1 - c0, 512):
                t1 = min(t0 + 512, c1 - c0)
                for m in range(0, KCH, 2):
                    nc.tensor.matmul(
                        tgt[:, t0:t1],
                        colbuf[:, m : m + 2, row * P : (row + 1) * P],
                        colbuf[:, m : m + 2, c0 + t0 : c0 + t1],
                        start=(m == 0),
                        stop=(m == KCH - 2),
                        perf_mode=DR,
                    )

        # ---- triangle job (own group, pairs a<=b) ----
        ps_tri = psum_p.tile([P, 1536], fp32, tag="ps", name="ps_tri")
        for a in range(NV):
            chain(ps_tri[:, TRI_OFF[a] : TRI_OFF[a] + TRI_W[a]], a, a * P, 512)
        e_tri = consts.tile([P, 1280], bf16, tag="e_tri")
        nc.scalar.activation(
            out=e_tri, in_=ps_tri[:, 0:1280], func=ACTF.Exp, scale=INV_T
        )
        # per-row rowsums of the triangle (DVE)
        for a in range(NV):
            nc.vector.reduce_sum(
                out=rsums[:, 4 + a : 5 + a],
                in_=e_tri[:, TRI_OFF[a] : TRI_OFF[a] + TRI_W[a]],
                axis=AX.X,
            )
        # same-sample diagonal entries (pos terms)
        for idx, (a, bb) in enumerate(DPAIRS):
            o = TRI_OFF[a] + (bb - a) * P
            dscr = dscr_p.tile([P, P], bf16, tag="dscr", name="dscr")
            nc.vector.scalar_tensor_tensor(
                out=dscr, in0=e_tri[:, o : o + P], scalar=1.0, in1=eye_sb,
                op0=ALU.mult, op1=ALU.mult,
                accum_out=dvals[:, idx : idx + 1],
            )
        # own-group column sums from the triangle's b>a blocks
        caccown = acc_p.tile([P, 384], bf16, tag="caccown")
        nc.vector.tensor_copy(out=caccown, in_=e_tri[:, 128:512])
        nc.vector.tensor_add(caccown[:, 128:384], caccown[:, 128:384],
                             e_tri[:, 640:896])
        nc.vector.tensor_add(caccown[:, 256:384], caccown[:, 256:384],
                             e_tri[:, 1152:1280])

        # ---- diameter-half blocks (need only own + partner pieces, which
        # load first: this fills the activation engine before the out-groups
        # arrive) ----
        cacch = acc_p.tile([P, 2, 256], bf16, tag="cacch")
        ps_half = psumh_p.tile([P, 1024], fp32, tag="psh", name="ps_half")
        ehalf = consts.tile([P, NV, 256], bf16, tag="ehalf")
        for k in range(NV):
            hs = 2048 if k < 2 else 2304
            chain(ps_half[:, k * 256 : (k + 1) * 256], k, hs, hs + 256)
        for k in range(NV):
            nc.scalar.activation(
                out=ehalf[:, k, :], in_=ps_half[:, k * 256 : (k + 1) * 256],
                func=ACTF.Exp, scale=INV_T,
                accum_out=rsums[:, 8 + k : 9 + k],
            )
        nc.vector.tensor_add(cacch[:, 0, :], ehalf[:, 0, :], ehalf[:, 1, :])
        nc.vector.tensor_add(cacch[:, 1, :], ehalf[:, 2, :], ehalf[:, 3, :])

        # ---- cross jobs k = 0..3 over the three out-groups ----
        caccg = acc_p.tile([P, 3, 512], bf16, tag="caccg")
        etiles = {}
        for k in range(NV):
            ps = psum_p.tile([P, 1536], fp32, tag="ps", name="psc")
            chain(ps, k, 512, 2048)
            ek = esb_p.tile([P, 1536], bf16, tag="ek", name="ek")
            nc.scalar.activation(
                out=ek, in_=ps, func=ACTF.Exp, scale=INV_T,
                accum_out=rsums[:, k : k + 1],
            )
            etiles[k] = ek
            # accumulate only E0..E2; E3's column contribution joins via a
            # chained ones-matmul / separate Pool reduce so nothing but the
            # final reductions trail the last activation
            if k == 1:
                for j in range(3):
                    nc.vector.tensor_add(
                        caccg[:, j, :],
                        etiles[0][:, j * 512 : (j + 1) * 512],
                        etiles[1][:, j * 512 : (j + 1) * 512],
                    )
            elif k == 2:
                for j in range(3):
                    nc.vector.tensor_add(
                        caccg[:, j, :], caccg[:, j, :],
                        etiles[k][:, j * 512 : (j + 1) * 512],
                    )

        # ---- column sums: halves + own + group o3 on gpsimd, groups
        # o1/o2 as ones-matmuls on the (idle by then) PE ----
        nc.gpsimd.partition_all_reduce(
            cspack[:, 0:256], cacch[:, 0, :], channels=P,
            reduce_op=bass_isa.ReduceOp.add,
        )
        nc.gpsimd.partition_all_reduce(
            cspack[:, 256:512], cacch[:, 1, :], channels=P,
            reduce_op=bass_isa.ReduceOp.add,
        )
        nc.gpsimd.partition_all_reduce(
            cspack[:, 512:896], caccown, channels=P,
            reduce_op=bass_isa.ReduceOp.add,
        )
        nc.gpsimd.partition_all_reduce(
            cspack[:, 896:1408], caccg[:, 2, :], channels=P,
            reduce_op=bass_isa.ReduceOp.add,
        )
        nc.gpsimd.partition_all_reduce(
            cspack[:, 1408:1920], etiles[3][:, 1024:1536], channels=P,
            reduce_op=bass_isa.ReduceOp.add,
        )
        cs2_ps = psumh_p.tile([P, 1024], fp32, tag="psh", name="cs2_ps")
        for j in range(2):
            nc.tensor.matmul(
                cs2_ps[0:1, j * 512 : (j + 1) * 512],
                ones1, caccg[:, j, :], start=True, stop=False,
            )
            nc.tensor.matmul(
                cs2_ps[0:1, j * 512 : (j + 1) * 512],
                ones1, etiles[3][:, j * 512 : (j + 1) * 512],
                start=False, stop=True,
            )
        # export the PE-side colsums into cspack so ONE tail DMA ships all
        # column sums (two serialized output DMAs cost ~1.2us of SP.SEQ)
        nc.scalar.copy(out=cspack[0:1, 1920:2432], in_=cs2_ps[0:1, 0:512])
        nc.vector.tensor_copy(
            out=cspack[0:1, 2432:2944], in_=cs2_ps[0:1, 512:1024]
        )

        nc.sync.dma_start(out=rs_d, in_=rsums)
        nc.sync.dma_start(out=dv_d, in_=dvals)
        nc.sync.dma_start(out=cs_d, in_=cspack[0:1, :])

    nc.compile()
    return nc


def _get_nc():
    if "nc" not in _compiled:
        _compiled["nc"] = _build_kernel()
    return _compiled["nc"]


def _core_slabs(q):
    """Per-core slab order: own group, 3 out-groups, partner halves."""
    order = [(v, q) for v in range(NV)]
    for d in (1, 2, 3):
        order += [(v, (q + d) % G) for v in range(NV)]
    p = (q + 4) % G
    halves = [0, 1, 2, 3] if q < 4 else [2, 3, 0, 1]
    order += [(v, p) for v in halves]
    return order


def _make_in_maps(english, etok, ktoe, korean, cs_ratios):
    V4f = np.stack([
        np.asarray(english, dtype=np.float32),
        np.asarray(korean, dtype=np.float32),
        np.asarray(etok, dtype=np.float32),
        np.asarray(ktoe, dtype=np.float32),
    ])  # [4, B, D], reference version order [e, k, etk, kte]
    V4f = V4f / np.linalg.norm(V4f, axis=2, keepdims=True)
    QT = np.ascontiguousarray(V4f.transpose(0, 2, 1)).astype(
        ml_dtypes.float8_e4m3
    )  # [4, D, B]

    in_maps = []
    for q in range(NC_CORES):
        cbuf = np.empty((P, KCH, WTOT), dtype=ml_dtypes.float8_e4m3)
        for j, (v, g) in enumerate(_core_slabs(q)):
            blk = QT[v][:, g * P : (g + 1) * P]  # [D, 128]
            cbuf[:, :, j * P : (j + 1) * P] = blk.reshape(
                KCH, P, P
            ).transpose(1, 0, 2)
        in_maps.append({"cb": cbuf.reshape(P, KCH * WTOT)})
    return in_maps, V4f


def kernel(english, etok, ktoe, korean, cs_ratios):
    from concourse.bass_utils import run_bass_kernel_spmd

    in_maps, V4f = _make_in_maps(english, etok, ktoe, korean, cs_ratios)
    nc = _get_nc()
    res = run_bass_kernel_spmd(nc, in_maps, core_ids=list(range(NC_CORES)))

    R = np.zeros((NV, B), dtype=np.float64)  # full-row sums per anchor
    dall = np.zeros((NV, NV, B), dtype=np.float64)  # same-sample exp entries
    for q, rmap in enumerate(res.results):
        rs = rmap["rs"].astype(np.float64)
        dv = rmap["dv"].astype(np.float64)
        cs = rmap["cs"].astype(np.float64)[0]
        cs2 = cs[1920:2944]
        sl = slice(q * P, (q + 1) * P)
        for k in range(NV):
            # cross main + triangle + diameter-half rowsum partials
            R[k, sl] += rs[:, k] + rs[:, 4 + k] + rs[:, 8 + k]
        for idx, (a, bb) in enumerate(DPAIRS):
            dall[a, bb, q * P : (q + 1) * P] = dv[:, idx]
            dall[bb, a, q * P : (q + 1) * P] = dv[:, idx]
        # column-sum contributions -> mirrored rows
        for d in (1, 2, 3):
            g = (q + d) % G
            piece = (cs2[(d - 1) * 512 : d * 512] if d < 3
                     else cs[896:1408] + cs[1408:1920])
            for v in range(NV):
                R[v, g * P : (g + 1) * P] += piece[v * P : (v + 1) * P]
        slabs = _core_slabs(q)
        ph = slabs[16:20]  # partner halves in core order
        for hh in range(2):
            piece = cs[hh * 256 : (hh + 1) * 256]
            for i in range(2):
                v, g = ph[hh * 2 + i]
                R[v, g * P : (g + 1) * P] += piece[i * P : (i + 1) * P]
        own = cs[512:896]
        for bb in range(1, NV):
            R[bb, sl] += own[(bb - 1) * P : bb * P]

    same_i = dall.sum(axis=1)  # [4, B]
    self_e = np.einsum("aab->ab", dall)
    pos = same_i - self_e
    tot = R - self_e  # pos + neg
    contrastive = (np.log(tot) - np.log(pos)).sum()

    r = np.asarray(cs_ratios, dtype=np.float64)[:, None]
    e, k64, etk, kte = (V4f[i].astype(np.float64) for i in range(4))
    reg = (
        np.linalg.norm(etk - (r * e + (1 - r) * k64), axis=1)
        + np.linalg.norm(kte - ((1 - r) * e + r * k64), axis=1)
    ).sum()

    return np.float32((contrastive + 0.5 * reg) / B)


# revision 5
# speedup vs baseline: 1.0679x; 1.0123x over previous
"""CodeSwitchLoss Trainium2 kernel, v2: symmetric-half similarity matrix.

E = exp(S/t) is symmetric, so only the upper-triangular block pairs are
computed (~half the matmul + exp area of the data-parallel baseline). Each
computed block contributes its row-sums (activation accumulator) to its row
anchors and its column-sums (DVE column accumulation + gpsimd partition
all-reduce) to its mirrored row anchors. Per-anchor partial sums, the
same-sample diagonal entries (pos terms), and the column sums are DMA'd out;
the tiny nonlinear combine (logs + cs regularization) runs host-side in
fp64, exactly as the baseline already ran normalization host-side.

Work layout (uniform across cores; per-core differences live in host data
packing only): 8 sample-groups of 128; slabs = (version, group). Core q owns
group q: a triangle job over its own group (pairs a<=b), four cross jobs
(row slab a of q) x [3 following groups + a 256-wide half of the diametric
partner group]. A circulant tournament (+1,+2,+3) plus the half-split of the
diameter class covers each of the 528 unordered block pairs exactly once.
"""

import numpy as np
import ml_dtypes

B = 1024
D = 1024
P = 128
NV = 4
G = 8
NC_CORES = 8
KCH = D // P  # 8
WTOT = 20 * P  # 2560 cols: own(4) + 3 groups(12) + partner(4) slabs
INV_T = 10.0

CROSS_W = 1792  # 1536 (3 groups) + 256 (diameter half)
# triangle row offsets: rows packed 0,1,3,2 so no block crosses a 512-col
# PSUM bank boundary (row2's 256-wide block would straddle col 1024)
TRI_OFF = [0, 512, 1024, 896]
TRI_W = [512, 384, 256, 128]
DPAIRS = [(a, b) for a in range(NV) for b in range(a, NV)]  # 10

_compiled = {}


def _build_kernel():
    from contextlib import ExitStack

    import concourse.tile as tile
    from concourse import bacc, bass_isa, mybir

    fp32 = mybir.dt.float32
    bf16 = mybir.dt.bfloat16
    int32 = mybir.dt.int32
    fp8 = mybir.dt.float8e4
    AX = mybir.AxisListType
    ALU = mybir.AluOpType
    ACTF = mybir.ActivationFunctionType
    DR = mybir.MatmulPerfMode.DoubleRow

    nc = bacc.Bacc(
        "TRN2",
        target_bir_lowering=False,
        debug=False,
        enable_asserts=False,
        num_devices=NC_CORES,
    )
    cb_d = nc.dram_tensor("cb", [P, KCH * WTOT], fp8, kind="ExternalInput").ap()
    rs_d = nc.dram_tensor("rs", [P, 12], fp32, kind="ExternalOutput").ap()
    dv_d = nc.dram_tensor("dv", [P, 10], fp32, kind="ExternalOutput").ap()
    cs_d = nc.dram_tensor("cs", [1, 2944], fp32, kind="ExternalOutput").ap()

    cbr = cb_d.rearrange("p (m w) -> p m w", m=KCH)

    with tile.TileContext(nc) as tc, ExitStack() as ctx:
        consts = ctx.enter_context(tc.tile_pool(name="consts", bufs=1))
        wpool = ctx.enter_context(tc.tile_pool(name="w", bufs=1))
        # PSUM budget (8 banks): psum_p 2 x [P,1536] (3 banks each) +
        # psumh_p 1 x [P,1024] (2 banks)
        psum_p = ctx.enter_context(tc.tile_pool(name="psum", bufs=2, space="PSUM"))
        psumh_p = ctx.enter_context(tc.tile_pool(name="psumh", bufs=1, space="PSUM"))
        esb_p = ctx.enter_context(tc.tile_pool(name="esb", bufs=4))
        acc_p = ctx.enter_context(tc.tile_pool(name="acc", bufs=1))
        dscr_p = ctx.enter_context(tc.tile_pool(name="dscr", bufs=2))

        # eye mask (bf16) built on device
        eye_i = consts.tile([P, P], int32, tag="eye_i")
        nc.gpsimd.iota(eye_i, pattern=[[1, P]], base=0, channel_multiplier=-1)
        eye_sb = consts.tile([P, P], bf16, tag="eye")
        nc.vector.tensor_scalar(
            out=eye_sb, in0=eye_i, scalar1=0, scalar2=None, op0=ALU.is_equal
        )

        # warm the Exp activation table at t=0 (only table we ever need)
        aw = consts.tile([P, 1], fp32, tag="actwarm")
        nc.vector.memset(aw, 0.0)
        nc.scalar.activation(out=aw, in_=aw, func=ACTF.Exp)

        # PE p-state keep-alive during the DMA staircase (paced on Pool so
        # DVE stays free for the real reduction work)
        wsb = consts.tile([P, P], bf16, tag="wsb")
        nc.vector.memset(wsb, 0.0)
        ones1 = consts.tile([P, 1], bf16, tag="ones1")
        nc.vector.memset(ones1, 1.0)
        ka_ps = psum_p.tile([P, 1536], fp32, tag="ps", name="ka_ps")
        nc.tensor.matmul(ka_ps[:, 0:16], wsb, wsb[:, 0:16], start=True, stop=True)
        ka_big = consts.tile([P, 900], fp32, tag="ka_big")
        ka_mv = consts.tile([P, 4, 4], bf16, tag="ka_mv")
        for kk in range(4):
            nc.gpsimd.memset(ka_big, 0.0)
            nc.gpsimd.memset(ka_mv[:, kk, :], 0.0)
            nc.tensor.matmul(
                ka_ps[:, 0:4], wsb, ka_mv[:, kk, :], start=True, stop=True
            )

        # ---- loads: own group, the 3 out-groups, partner halves last (the
        # halves are computed with a DVE fast-exp, off the ACT critical path,
        # so the big cross activations start ~1.5us earlier) ----
        colbuf = wpool.tile([P, KCH, WTOT], fp8, tag="colbuf")
        for s in (0, 512, 1024, 1536, 2048):
            nc.sync.dma_start(
                out=colbuf[:, :, s : s + 512], in_=cbr[:, :, s : s + 512]
            )

        rsums = acc_p.tile([P, 12], fp32, tag="rsums")
        dvals = acc_p.tile([P, 10], fp32, tag="dvals")
        cspack = acc_p.tile([P, 2944], fp32, tag="cspack")

        def chain(tgt, row, c0, c1):
            """PSUM-accumulating fp8 DoubleRow chains: rows = slab `row`,
            cols = colbuf[c0:c1], tiled to the 512-col PSUM-bank limit."""
            for t0 in range(0, c1 - c0, 512):
                t1 = min(t0 + 512, c1 - c0)
                for m in range(0, KCH, 2):
                    nc.tensor.matmul(
                        tgt[:, t0:t1],
                        colbuf[:, m : m + 2, row * P : (row + 1) * P],
                        colbuf[:, m : m + 2, c0 + t0 : c0 + t1],
                        start=(m == 0),
                        stop=(m == KCH - 2),
                        perf_mode=DR,
                    )

        # ---- triangle job (own group, pairs a<=b) ----
        ps_tri = psum_p.tile([P, 1536], fp32, tag="ps", name="ps_tri")
        for a in range(NV):
            chain(ps_tri[:, TRI_OFF[a] : TRI_OFF[a] + TRI_W[a]], a, a * P, 512)
        e_tri = consts.tile([P, 1280], bf16, tag="e_tri")
        nc.scalar.activation(
            out=e_tri, in_=ps_tri[:, 0:1280], func=ACTF.Exp, scale=INV_T
        )
        # per-row rowsums of the triangle (DVE)
        for a in range(NV):
            nc.vector.reduce_sum(
                out=rsums[:, 4 + a : 5 + a],
                in_=e_tri[:, TRI_OFF[a] : TRI_OFF[a] + TRI_W[a]],
                axis=AX.X,
            )
        # same-sample diagonal entries (pos terms)
        for idx, (a, bb) in enumerate(DPAIRS):
            o = TRI_OFF[a] + (bb - a) * P
            dscr = dscr_p.tile([P, P], bf16, tag="dscr", name="dscr")
            nc.vector.scalar_tensor_tensor(
                out=dscr, in0=e_tri[:, o : o + P], scalar=1.0, in1=eye_sb,
                op0=ALU.mult, op1=ALU.mult,
                accum_out=dvals[:, idx : idx + 1],
            )
        # own-group column sums from the triangle's b>a blocks
        caccown = acc_p.tile([P, 384], bf16, tag="caccown")
        nc.vector.tensor_copy(out=caccown, in_=e_tri[:, 128:512])
        nc.vector.tensor_add(caccown[:, 128:384], caccown[:, 128:384],
                             e_tri[:, 640:896])
        nc.vector.tensor_add(caccown[:, 256:384], caccown[:, 256:384],
                             e_tri[:, 1152:1280])

        cacch = acc_p.tile([P, 2, 256], bf16, tag="cacch")

        def emit_halves():
            """Diameter-half blocks. exp via Schraudolph fast-exp on DVE:
            bf16 bit pattern z = s*1280*log2(e) + (127*128 - C) built as a
            rounded int16, bitcast to bf16. Halves are cross-chunk blocks
            (|s| < ~0.4, no pos-term diagonals) so the ~1% sawtooth error
            only perturbs large negative-sum aggregates."""
            ps_half = psumh_p.tile([P, 1024], fp32, tag="psh", name="ps_half")
            ehb = consts.tile([P, NV, 256], mybir.dt.int16, tag="ehb")
            for kk in range(NV):
                hs = 2048 if kk < 2 else 2304
                chain(ps_half[:, kk * 256 : (kk + 1) * 256], kk, hs, hs + 256)
            for kk in range(NV):
                nc.vector.tensor_scalar(
                    out=ehb[:, kk, :],
                    in0=ps_half[:, kk * 256 : (kk + 1) * 256],
                    scalar1=1846.6496, scalar2=16250.24,
                    op0=ALU.mult, op1=ALU.add,
                )
                nc.vector.reduce_sum(
                    out=rsums[:, 8 + kk : 9 + kk],
                    in_=ehb[:, kk, :].bitcast(bf16), axis=AX.X,
                )
            nc.vector.tensor_add(
                cacch[:, 0, :], ehb[:, 0, :].bitcast(bf16),
                ehb[:, 1, :].bitcast(bf16),
            )
            nc.vector.tensor_add(
                cacch[:, 1, :], ehb[:, 2, :].bitcast(bf16),
                ehb[:, 3, :].bitcast(bf16),
            )

        # ---- cross jobs k = 0..3 over the three out-groups ----
        caccg = acc_p.tile([P, 3, 512], bf16, tag="caccg")
        etiles = {}
        for k in range(NV):
            ps = psum_p.tile([P, 1536], fp32, tag="ps", name="psc")
            chain(ps, k, 512, 2048)
            ek = esb_p.tile([P, 1536], bf16, tag="ek", name="ek")
            nc.scalar.activation(
                out=ek, in_=ps, func=ACTF.Exp, scale=INV_T,
                accum_out=rsums[:, k : k + 1],
            )
            etiles[k] = ek
            # accumulate only E0..E2; E3's column contribution joins via a
            # chained ones-matmul / separate Pool reduce so nothing but the
            # final reductions trail the last activation
            if k == 1:
                for j in range(3):
                    nc.vector.tensor_add(
                        caccg[:, j, :],
                        etiles[0][:, j * 512 : (j + 1) * 512],
                        etiles[1][:, j * 512 : (j + 1) * 512],
                    )
                # halves here: their PE chains slot between k1 and k2 (all
                # cross data is resident by the time partner lands)
                emit_halves()
            elif k == 2:
                for j in range(3):
                    nc.vector.tensor_add(
                        caccg[:, j, :], caccg[:, j, :],
                        etiles[k][:, j * 512 : (j + 1) * 512],
                    )

        # ---- column sums: halves + own + group o3 on gpsimd, groups
        # o1/o2 as ones-matmuls on the (idle by then) PE ----
        nc.gpsimd.partition_all_reduce(
            cspack[:, 0:256], cacch[:, 0, :], channels=P,
            reduce_op=bass_isa.ReduceOp.add,
        )
        nc.gpsimd.partition_all_reduce(
            cspack[:, 256:512], cacch[:, 1, :], channels=P,
            reduce_op=bass_isa.ReduceOp.add,
        )
        nc.gpsimd.partition_all_reduce(
            cspack[:, 512:896], caccown, channels=P,
            reduce_op=bass_isa.ReduceOp.add,
        )
        nc.gpsimd.partition_all_reduce(
            cspack[:, 896:1408], caccg[:, 2, :], channels=P,
            reduce_op=bass_isa.ReduceOp.add,
        )
        nc.gpsimd.partition_all_reduce(
            cspack[:, 1408:1920], etiles[3][:, 1024:1536], channels=P,
            reduce_op=bass_isa.ReduceOp.add,
        )
        cs2_ps = psumh_p.tile([P, 1024], fp32, tag="psh", name="cs2_ps")
        for j in range(2):
            nc.tensor.matmul(
                cs2_ps[0:1, j * 512 : (j + 1) * 512],
                ones1, caccg[:, j, :], start=True, stop=False,
            )
            nc.tensor.matmul(
                cs2_ps[0:1, j * 512 : (j + 1) * 512],
                ones1, etiles[3][:, j * 512 : (j + 1) * 512],
                start=False, stop=True,
            )
        # export the PE-side colsums into cspack so ONE tail DMA ships all
        # column sums (two serialized output DMAs cost ~1.2us of SP.SEQ)
        nc.scalar.copy(out=cspack[0:1, 1920:2432], in_=cs2_ps[0:1, 0:512])
        nc.vector.tensor_copy(
            out=cspack[0:1, 2432:2944], in_=cs2_ps[0:1, 512:1024]
        )

        nc.sync.dma_start(out=rs_d, in_=rsums)
        nc.sync.dma_start(out=dv_d, in_=dvals)
        nc.sync.dma_start(out=cs_d, in_=cspack[0:1, :])

    nc.compile()
    return nc


def _get_nc():
    if "nc" not in _compiled:
        _compiled["nc"] = _build_kernel()
    return _compiled["nc"]


def _core_slabs(q):
    """Per-core slab order: own group, 3 out-groups, partner halves."""
    order = [(v, q) for v in range(NV)]
    for d in (1, 2, 3):
        order += [(v, (q + d) % G) for v in range(NV)]
    p = (q + 4) % G
    halves = [0, 1, 2, 3] if q < 4 else [2, 3, 0, 1]
    order += [(v, p) for v in halves]
    return order


def _make_in_maps(english, etok, ktoe, korean, cs_ratios):
    V4f = np.stack([
        np.asarray(english, dtype=np.float32),
        np.asarray(korean, dtype=np.float32),
        np.asarray(etok, dtype=np.float32),
        np.asarray(ktoe, dtype=np.float32),
    ])  # [4, B, D], reference version order [e, k, etk, kte]
    V4f = V4f / np.linalg.norm(V4f, axis=2, keepdims=True)
    QT = np.ascontiguousarray(V4f.transpose(0, 2, 1)).astype(
        ml_dtypes.float8_e4m3
    )  # [4, D, B]

    in_maps = []
    for q in range(NC_CORES):
        cbuf = np.empty((P, KCH, WTOT), dtype=ml_dtypes.float8_e4m3)
        for j, (v, g) in enumerate(_core_slabs(q)):
            blk = QT[v][:, g * P : (g + 1) * P]  # [D, 128]
            cbuf[:, :, j * P : (j + 1) * P] = blk.reshape(
                KCH, P, P
            ).transpose(1, 0, 2)
        in_maps.append({"cb": cbuf.reshape(P, KCH * WTOT)})
    return in_maps, V4f


def kernel(english, etok, ktoe, korean, cs_ratios):
    from concourse.bass_utils import run_bass_kernel_spmd

    in_maps, V4f = _make_in_maps(english, etok, ktoe, korean, cs_ratios)
    nc = _get_nc()
    res = run_bass_kernel_spmd(nc, in_maps, core_ids=list(range(NC_CORES)))

    R = np.zeros((NV, B), dtype=np.float64)  # full-row sums per anchor
    dall = np.zeros((NV, NV, B), dtype=np.float64)  # same-sample exp entries
    for q, rmap in enumerate(res.results):
        rs = rmap["rs"].astype(np.float64)
        dv = rmap["dv"].astype(np.float64)
        cs = rmap["cs"].astype(np.float64)[0]
        cs2 = cs[1920:2944]
        sl = slice(q * P, (q + 1) * P)
        for k in range(NV):
            # cross main + triangle + diameter-half rowsum partials
            R[k, sl] += rs[:, k] + rs[:, 4 + k] + rs[:, 8 + k]
        for idx, (a, bb) in enumerate(DPAIRS):
            dall[a, bb, q * P : (q + 1) * P] = dv[:, idx]
            dall[bb, a, q * P : (q + 1) * P] = dv[:, idx]
        # column-sum contributions -> mirrored rows
        for d in (1, 2, 3):
            g = (q + d) % G
            piece = (cs2[(d - 1) * 512 : d * 512] if d < 3
                     else cs[896:1408] + cs[1408:1920])
            for v in range(NV):
                R[v, g * P : (g + 1) * P] += piece[v * P : (v + 1) * P]
        slabs = _core_slabs(q)
        ph = slabs[16:20]  # partner halves in core order
        for hh in range(2):
            piece = cs[hh * 256 : (hh + 1) * 256]
            for i in range(2):
                v, g = ph[hh * 2 + i]
                R[v, g * P : (g + 1) * P] += piece[i * P : (i + 1) * P]
        own = cs[512:896]
        for bb in range(1, NV):
            R[bb, sl] += own[(bb - 1) * P : bb * P]

    same_i = dall.sum(axis=1)  # [4, B]
    self_e = np.einsum("aab->ab", dall)
    pos = same_i - self_e
    tot = R - self_e  # pos + neg
    contrastive = (np.log(tot) - np.log(pos)).sum()

    r = np.asarray(cs_ratios, dtype=np.float64)[:, None]
    e, k64, etk, kte = (V4f[i].astype(np.float64) for i in range(4))
    reg = (
        np.linalg.norm(etk - (r * e + (1 - r) * k64), axis=1)
        + np.linalg.norm(kte - ((1 - r) * e + r * k64), axis=1)
    ).sum()

    return np.float32((contrastive + 0.5 * reg) / B)
